# revision 42
# baseline (speedup 1.0000x reference)
"""Causal scaled-dot-product attention for Trainium2 (Bass/Tile), 8-core SPMD.

Problem: B=2, H=16, S=2048, D=128 fp32, causal mask, softmax(QK^T/sqrt(D)) @ V.
Sharding: batch*heads (32) split across 8 cores, 4 heads per core; attention is
independent per (b,h): no communication.

Design (90.4us measured vs the 115.3us v1 baseline; rel err 9.8e-3):
  - All layout/dtype prep host-side. Q,K ship transposed [D,S] bf16 (fp8
    gives no PE speedup on TRN2 -- 1 col/cycle either way -- so bf16 keeps
    the accuracy for free); V ships fp8e4m3 partition-major plus a bf16 copy
    of its first 256 rows. Q/K DMAs are split into 512-column pieces so the
    first QK matmul can start as soon as ~256KB has landed.
  - exp is split across two engines so the Scalar engine never paces the
    pipeline:
      * most full (non-diagonal) pairs: ACT exp -> fp8e4m3 (exact path)
      * diagonal pairs + every 3rd full pair: DVE tensor_scalar computes
          y_int8 = round(psum * (4*log2e/T) + bias)
        and the int8 bytes ARE fp8e5m2 exp values (Schraudolph bit-trick,
        4 bytes/octave; the fp32->int8 convert rounds-to-nearest and
        saturates on HW). For diagonal pairs the bias comes from a constant
        maskbias tile: the exp bias on valid positions, -1000 on causally
        masked + stale positions, which saturates to int8 -128 = e5m2
        "-0.0" (harmless in the PV/den matmuls). One DVE op = exp + causal
        mask + stale kill. e5m2's byte window spans ~22 z-units: no wrap
        cliffs for any input.
  - PV per pair: ONE fp8 DoubleRow matmul (contraction 256), full width
    from the pair's first valid column -- masked/stale entries are -0.0 so
    no strip matmuls are needed. Mixed e4m3 weights x e5m2 moving verified
    on HW.
  - den matmuls for a whole chunk are emitted as one deferred batch (two
    pairs into the next chunk): consecutive DoubleRow matmuls sharing the
    constant ones weights stream at 1 col/cycle, where fresh-weight
    LDWEIGHTS cost ~190ns extra each (256-row DR weight loads do not
    double-buffer).
  - PSUM: ps_s [128,1024]x3 (6 banks) + ps_o [128,512] + ps_d [1,512].
    The 3-deep ps_s ring lets QK(g) proceed once exp(g-3) is done, which
    both deepens the HW pipeline and lets the Tile scheduler keep the PE
    stream dense.
  - No on-device softmax normalization: the kernel ships OUT^T (bf16,
    unnormalized) and den (f32); the host divides. This removes the fp32
    broadcast matmuls, the reciprocal chain, and the PE stalls at each
    chunk tail.

Numerics: softmax shift exp(z/T - 2) keeps the exact-path exp <= ~53 (no fp8
clipping); numerator and denominator consume the same quantized P~, so P
quantization largely cancels in the host-side normalization. First key-tile
pair of each head runs in bf16 (rows with <256 keys get no averaging of V's
fp8 quantization error). Measured worst rel err 9.8e-3 (tol 2e-2).
"""
import numpy as np

import concourse.bacc as bacc
import concourse.tile as tile
import concourse.mybir as mybir
from concourse.bass_utils import run_bass_kernel_spmd
from concourse.masks import make_upper_triangular

F32 = mybir.dt.float32
BF16 = mybir.dt.bfloat16
F8 = mybir.dt.float8e4
E5 = mybir.dt.float8e5
I8 = mybir.dt.int8
EXP = mybir.ActivationFunctionType.Exp
DR = mybir.MatmulPerfMode.DoubleRow

B, H, S, D = 2, 16, 2048, 128
TEMPERATURE = 11.313708498984761  # sqrt(128)
EXP_BIAS = -2.0          # exp(z/temp - 2): keeps exact-path exp <= ~53
A5 = 5.770780163555855   # 4*log2(e): e5m2 bytes per ln unit
C5 = 0.25                # Schraudolph round-to-nearest correction (tuned)
SCALE5 = A5 / TEMPERATURE
MB_VALID = 60.0 - C5 + A5 * EXP_BIAS   # fast-exp byte bias on valid entries
MB_MASK = -1000.0                      # masked -> int8 -128 -> e5m2 -0.0
N_CORES = 8
HEADS_PER_CORE = (B * H) // N_CORES  # 4
P = 128
CHUNK = 512
N_KT = S // P              # 16 key tiles per head
N_CH = S // CHUNK          # 4 query chunks per head


def build_attention_nc():
    nc = bacc.Bacc("TRN2", target_bir_lowering=False, debug=False,
                   num_devices=N_CORES)
    qT_d = nc.dram_tensor("qT", [HEADS_PER_CORE, D, S], BF16,
                          kind="ExternalInput").ap()
    kT_d = nc.dram_tensor("kT", [HEADS_PER_CORE, D, S], BF16,
                          kind="ExternalInput").ap()
    v8_d = nc.dram_tensor("v8", [HEADS_PER_CORE, P, N_KT, P], F8,
                          kind="ExternalInput").ap()
    vb_d = nc.dram_tensor("vb", [HEADS_PER_CORE, P, 2, P], BF16,
                          kind="ExternalInput").ap()
    mba_d = nc.dram_tensor("mba", [P, 2 * CHUNK], F32,
                           kind="ExternalInput").ap()
    mbb_d = nc.dram_tensor("mbb", [P, 2 * CHUNK], F32,
                           kind="ExternalInput").ap()
    o_d = nc.dram_tensor("oT", [HEADS_PER_CORE, D, S], BF16,
                         kind="ExternalOutput").ap()
    den_d = nc.dram_tensor("den", [HEADS_PER_CORE, N_CH, CHUNK], F32,
                           kind="ExternalOutput").ap()

    with tile.TileContext(nc) as tc:
        with tc.tile_pool(name="sb", bufs=1) as sb, \
             tc.tile_pool(name="ps_s", bufs=3, space="PSUM") as ps_s, \
             tc.tile_pool(name="ps_o", bufs=1, space="PSUM") as ps_o, \
             tc.tile_pool(name="ps_d", bufs=1, space="PSUM") as ps_d:
            consts = qkt = px = sm = sb

            # ---- constants ----
            utm = consts.tile([P, P], BF16)  # utm[k,q] = 1 iff q >= k
            make_upper_triangular(nc, utm, val=1.0, diag=True)
            ones_col = consts.tile([P, 1], BF16)
            nc.vector.memset(ones_col, 1.0)
            # fp8 ones pair for DoubleRow den matmuls ([128,2,1], 16B-aligned
            # pair stride per the DoubleRow weight AP requirement)
            ones8w = consts.tile([P, 2, 16], F8)
            nc.vector.memset(ones8w, 1.0)
            ones8 = ones8w[:, :, 0:1]
            wscr = consts.tile([P, CHUNK], BF16)
            nc.vector.memset(wscr, 1.0)
            bias_ap = consts.tile([P, 1], F32)
            nc.vector.memset(bias_ap, EXP_BIAS)
            mba = consts.tile([P, 2 * CHUNK], F32)
            mbb = consts.tile([P, 2 * CHUNK], F32)
            # preload the ACT exp table during the head-0 DMA (the implicit
            # ACT_TABLE_LOAD takes ~1.3us and would otherwise stall the
            # first real exp)
            actwarm = consts.tile([P, 1], F8)
            nc.scalar.activation(actwarm, bias_ap, EXP, bias=0.0, scale=1.0)

            head_state = {}

            def emit_load(hh, first_head=False):
                h = hh % HEADS_PER_CORE
                # split Q/K into 512-col pieces so chunk-0 work can start
                # before the whole head has landed
                kt = [qkt.tile([P, CHUNK], BF16, tag=f"kt{i}", name=f"kt{i}",
                               bufs=2) for i in range(4)]
                qc = [qkt.tile([P, CHUNK], BF16, tag=f"qc{i}", name=f"qc{i}",
                               bufs=2) for i in range(4)]
                v8 = qkt.tile([P, N_KT, P], F8, tag="v8", name="v8", bufs=2)
                vb = qkt.tile([P, 2, P], BF16, tag="vb", name="vb", bufs=2)
                nc.sync.dma_start(out=kt[0], in_=kT_d[h, :, 0:CHUNK])
                nc.sync.dma_start(out=qc[0], in_=qT_d[h, :, 0:CHUNK])
                nc.sync.dma_start(out=vb, in_=vb_d[h])
                if first_head:
                    # mask-bias constants are first needed by pair (2,3)
                    nc.sync.dma_start(out=mba, in_=mba_d)
                    nc.sync.dma_start(out=mbb, in_=mbb_d)
                nc.sync.dma_start(out=v8, in_=v8_d[h])
                for i in range(1, 4):
                    nc.sync.dma_start(out=qc[i],
                                      in_=qT_d[h, :, CHUNK * i:CHUNK * (i + 1)])
                    nc.sync.dma_start(out=kt[i],
                                      in_=kT_d[h, :, CHUNK * i:CHUNK * (i + 1)])
                head_state[hh] = dict(kt=kt, qc=qc, v8=v8, vb=vb)

            emit_load(0, first_head=True)

            def emit_dummies(n):
                # real MAC activity to open the HAM clock gate / p-state
                # ramp. Covers BOTH ps_s ring slots over their full width so
                # every psum_s bit is initialized (bounded) before the
                # fast-exp path ever reads a stale region.
                for _ in range(n):
                    warm = ps_s.tile([P, 2 * CHUNK], F32, tag="psm",
                                     name="psm")
                    nc.tensor.matmul(warm[:, 0:CHUNK], wscr[:, 0:P], wscr,
                                     start=True, stop=True,
                                     skip_group_check=True)
                    nc.tensor.matmul(warm[:, CHUNK:2 * CHUNK], wscr[:, 0:P],
                                     wscr, start=True, stop=True,
                                     skip_group_check=True)

            def emit_pv_first(st, pexp, psum_o):
                # bf16 PV for the head's first pair (tiles 0,1): per-tile
                # matmuls with column offsets (skip the stale gap [512:640))
                for (j, oj) in ((0, 0), (1, P)):
                    base = j * CHUNK
                    nc.tensor.matmul(
                        psum_o[:, oj:CHUNK], st["vb"][:, j, :],
                        pexp[:, base + oj:base + CHUNK],
                        start=(j == 0), stop=False,
                        skip_group_check=True)

            def emit_pv(st, j0, oj0, pexp8, psum_o, start, stop):
                # one DoubleRow matmul pair over [oj0:CHUNK]; masked/stale
                # entries in pexp8 are (-)0.0 so the full width is safe
                p3 = pexp8.rearrange("p (a b) -> p a b", a=2)
                nc.tensor.matmul(
                    psum_o[:, oj0:CHUNK], st["v8"][:, j0:j0 + 2, :],
                    p3[:, :, oj0:CHUNK],
                    start=start, stop=stop,
                    perf_mode=DR, skip_group_check=True)

            def emit_chunk_pvs(st, chunk_pexps, psum_o):
                # all of a chunk's PV matmuls back-to-back: one PE
                # bf16<->fp8-DR mode transition per burst instead of two
                # per pair
                n = len(chunk_pexps)
                for i, (kind, pexp, oj0, j0) in enumerate(chunk_pexps):
                    if kind == "first":
                        emit_pv_first(st, pexp, psum_o)
                    else:
                        emit_pv(st, j0, oj0, pexp, psum_o,
                                start=(i == 0), stop=(i == n - 1))

            def emit_den_batch(chunk_pexps, psum_d, start=True, stop=True):
                # all of a chunk's den matmuls back-to-back: consecutive
                # DoubleRow matmuls sharing the constant ones weights stream
                # at 1 col/cycle (fresh-weight LDWEIGHTS would add ~190ns
                # per matmul otherwise)
                n = len(chunk_pexps)
                for i, (kind, pexp, oj0, j0) in enumerate(chunk_pexps):
                    if kind == "first":
                        for (j, oj) in ((0, 0), (1, P)):
                            base = j * CHUNK
                            nc.tensor.matmul(
                                psum_d[:, oj:CHUNK], ones_col,
                                pexp[:, base + oj:base + CHUNK],
                                start=(start and i == 0 and j == 0),
                                stop=False,
                                skip_group_check=True)
                    else:
                        p3 = pexp.rearrange("p (a b) -> p a b", a=2)
                        nc.tensor.matmul(
                            psum_d[:, oj0:CHUNK], ones8,
                            p3[:, :, oj0:CHUNK],
                            start=(start and i == 0),
                            stop=(stop and i == n - 1),
                            perf_mode=DR, skip_group_check=True)

            def make_tail(hh, c, psum_o, psum_d):
                def emit():
                    h = hh % HEADS_PER_CORE
                    outT = sm.tile([P, CHUNK], BF16, tag="outT", name="outT",
                                   bufs=3)
                    denb = sm.tile([1, CHUNK], F32, tag="denb", name="denb",
                                   bufs=3)
                    nc.scalar.copy(outT, psum_o)
                    nc.vector.tensor_copy(denb, psum_d)
                    nc.sync.dma_start(
                        out=o_d[h, :, CHUNK * c:CHUNK * (c + 1)], in_=outT)
                    nc.sync.dma_start(out=den_d[h, c:c + 1], in_=denb)
                return emit

            # ---- PE warm-up during the head-0 DMA ----
            # 3 iterations cover all 3 ps_s ring slots exactly
            emit_dummies(3)

            deferred = []           # FIFO of (due_group_idx, fn)
            group_idx = 0

            def pump(final=False):
                while deferred and (final or group_idx >= deferred[0][0]):
                    deferred.pop(0)[1]()

            def kw(st, j):
                # K^T weights for key tile j out of the split kT pieces
                return st["kt"][j // 4][:, (j % 4) * P:(j % 4 + 1) * P]

            for hh in range(HEADS_PER_CORE):
                st = head_state[hh]
                if hh + 1 < HEADS_PER_CORE:
                    emit_load(hh + 1)

                for c in range(N_CH):
                    last = (hh == HEADS_PER_CORE - 1 and c == N_CH - 1)
                    if last:
                        # clear the previous chunk's deferred den batch/tail
                        # before the eager last chunk touches psum_d/psum_o
                        pump(final=True)
                    jmax = 4 * c + 3
                    psum_o = ps_o.tile([P, CHUNK], F32, tag="po", name="po")
                    psum_d = ps_d.tile([1, CHUNK], F32, tag="pd", name="pd")
                    chunk_pexps = []

                    for jp in range(2 * c + 2):
                        j0 = 2 * jp
                        first = (c == 0 and jp == 0)
                        typeA = (j0 == 4 * c) and not first
                        typeB = (j0 == 4 * c + 2)
                        psum_s = ps_s.tile([P, 2 * CHUNK], F32, tag="psm",
                                           name="psm")

                        if first:
                            nc.tensor.matmul(
                                psum_s[:, 0:CHUNK], kw(st, 0), st["qc"][0],
                                start=True, stop=True)
                            nc.tensor.matmul(
                                psum_s[:, CHUNK + P:2 * CHUNK], kw(st, 1),
                                st["qc"][0][:, P:CHUNK],
                                start=True, stop=True)
                            pexp16 = px.tile([P, 2 * CHUNK], BF16,
                                             tag="pexp16", name="pexp16",
                                             bufs=2)
                            nc.scalar.activation(
                                pexp16, psum_s, EXP,
                                bias=bias_ap, scale=1.0 / TEMPERATURE)
                            # causal masks for the two diagonal blocks
                            nc.gpsimd.tensor_mul(
                                pexp16[:, 0:P], pexp16[:, 0:P], utm)
                            nc.gpsimd.tensor_mul(
                                pexp16[:, CHUNK + P:CHUNK + 2 * P],
                                pexp16[:, CHUNK + P:CHUNK + 2 * P], utm)
                            chunk_pexps.append(("first", pexp16, 0, 0))
                            if last:
                                emit_pv_first(st, pexp16, psum_o)
                        else:
                            oj0 = max(0, P * j0 - CHUNK * c)
                            oj1 = max(0, P * (j0 + 1) - CHUNK * c)
                            nc.tensor.matmul(
                                psum_s[:, oj0:CHUNK], kw(st, j0),
                                st["qc"][c][:, oj0:CHUNK],
                                start=True, stop=True)
                            nc.tensor.matmul(
                                psum_s[:, CHUNK + oj1:2 * CHUNK],
                                kw(st, j0 + 1),
                                st["qc"][c][:, oj1:CHUNK],
                                start=True, stop=True)
                            diag = typeA or typeB
                            if diag or jp % 3 == 2:
                                # DVE fast-exp -> e5m2 bytes; diagonal pairs
                                # add the fused causal mask via the maskbias
                                # tile, full pairs use an immediate bias
                                pexpd = px.tile([P, 2 * CHUNK], E5,
                                                tag="pexpd", name="pexpd",
                                                bufs=4)
                                pexp_i8 = pexpd.bitcast(I8)
                                if diag:
                                    mb = mba if typeA else mbb
                                    nc.vector.scalar_tensor_tensor(
                                        pexp_i8[:, oj0:2 * CHUNK],
                                        psum_s[:, oj0:2 * CHUNK], SCALE5,
                                        mb[:, oj0:2 * CHUNK],
                                        mybir.AluOpType.mult,
                                        mybir.AluOpType.add)
                                else:
                                    nc.vector.tensor_scalar(
                                        pexp_i8[:, 0:2 * CHUNK],
                                        psum_s[:, 0:2 * CHUNK], SCALE5,
                                        MB_VALID,
                                        mybir.AluOpType.mult,
                                        mybir.AluOpType.add)
                                pexp8 = pexpd
                            else:
                                # exact path: ACT exp -> fp8e4m3
                                pexp8 = px.tile([P, 2 * CHUNK], F8,
                                                tag="pexp8", name="pexp8",
                                                bufs=5)
                                nc.scalar.activation(
                                    pexp8, psum_s, EXP,
                                    bias=bias_ap, scale=1.0 / TEMPERATURE)
                            chunk_pexps.append(("pair", pexp8, oj0, j0))
                        group_idx += 1
                        pump()
                        if last and jp >= 1:
                            # last chunk: emit the PREVIOUS pair's PV+den
                            # (lag 1) so they never wait on their own exp,
                            # while keeping the final flush short
                            (_, ppexp, poj0, pj0) = chunk_pexps[-2]
                            emit_pv(st, pj0, poj0, ppexp, psum_o,
                                    start=(pj0 == 0), stop=False)
                            emit_den_batch(chunk_pexps[-2:-1], psum_d,
                                           start=(jp == 1), stop=False)

                    if last:
                        (_, lpexp, loj0, lj0) = chunk_pexps[-1]
                        emit_pv(st, lj0, loj0, lpexp, psum_o,
                                start=False, stop=True)
                        emit_den_batch(chunk_pexps[-1:], psum_d,
                                       start=False, stop=True)

                    # den batch + evac run two pairs into the NEXT chunk so
                    # the PE never waits on this chunk's last exp before
                    # starting the next chunk's QKs. With single-buffered
                    # ps_o/ps_d the evac MUST be emitted before the next
                    # chunk's first PV pops (at pair 3, lag 3) -- due+2 and
                    # FIFO order (batch, then tail) guarantee that.
                    pexps = list(chunk_pexps)
                    pd = psum_d
                    po = psum_o
                    if not last:
                        # (the last chunk emitted its PVs/dens per pair)
                        deferred.append((group_idx + 2,
                                         lambda ps=pexps, s=st, o=po, d=pd: (
                                             emit_chunk_pvs(s, ps, o),
                                             emit_den_batch(ps, d))))
                    deferred.append((group_idx + 3, make_tail(hh, c, psum_o,
                                                              psum_d)))

            pump(final=True)

    nc.compile()
    return nc


_NC_CACHE = None


def _get_nc():
    global _NC_CACHE
    if _NC_CACHE is None:
        _NC_CACHE = build_attention_nc()
    return _NC_CACHE


def _build_maskbias():
    """Constant [128,1024] f32 bias tiles for the two diagonal pair types.

    Pair layout: tile j0 at cols [0:512), tile j1 at cols [512:1024).
    Type A (oj0=0, oj1=128): masked at {col < p} in tile j0's diag block and
      cols [512, 640+p) (stale gap + tile j1 diag block).
    Type B (oj0=256, oj1=384): masked at cols [256, 256+p) and [512, 896+p).
    """
    pidx = np.arange(P)[:, None]
    cidx = np.arange(2 * CHUNK)[None, :]
    mba = np.where((cidx < pidx) | ((cidx >= 512) & (cidx < 640 + pidx)),
                   MB_MASK, MB_VALID).astype(np.float32)
    mbb = np.where((cidx < 256 + pidx) | ((cidx >= 512) & (cidx < 896 + pidx)),
                   MB_MASK, MB_VALID).astype(np.float32)
    return mba, mbb


def kernel(q, k, v, mask=None, _trace=False):
    """Full-input entry point: q,k,v [2,16,2048,128] f32, mask [2,1,2048,2048]
    int32 (causal; the kernel hardcodes causality and does not read it).
    Returns [2,16,2048,128] f32. Layout/dtype prep, the softmax
    normalization (out/den), and the inverse output transpose run host-side.
    """
    import ml_dtypes
    bf16 = ml_dtypes.bfloat16
    f8 = ml_dtypes.float8_e4m3fn

    nc = _get_nc()
    BH = B * H
    qf = np.asarray(q, dtype=np.float32).reshape(BH, S, D)
    kf = np.asarray(k, dtype=np.float32).reshape(BH, S, D)
    vf = np.asarray(v, dtype=np.float32).reshape(BH, S, D)
    qT = np.ascontiguousarray(qf.transpose(0, 2, 1)).astype(bf16)  # [BH,D,S]
    kT = np.ascontiguousarray(kf.transpose(0, 2, 1)).astype(bf16)
    # V partition-major: [BH, S, D] -> [BH, P, N_KT, D]
    v8 = np.ascontiguousarray(
        vf.reshape(BH, N_KT, P, D).transpose(0, 2, 1, 3)).astype(f8)
    vb = np.ascontiguousarray(
        vf[:, 0:2 * P].reshape(BH, 2, P, D).transpose(0, 2, 1, 3)).astype(bf16)
    mba, mbb = _build_maskbias()

    in_maps = []
    for i in range(N_CORES):
        sl = slice(i * HEADS_PER_CORE, (i + 1) * HEADS_PER_CORE)
        in_maps.append({"qT": qT[sl], "kT": kT[sl], "v8": v8[sl],
                        "vb": vb[sl], "mba": mba, "mbb": mbb})
    res = run_bass_kernel_spmd(nc, in_maps, list(range(N_CORES)), trace=_trace)
    oT = np.concatenate([res.results[i]["oT"] for i in range(N_CORES)],
                        axis=0)                        # [BH, D, S] bf16
    den = np.concatenate([res.results[i]["den"] for i in range(N_CORES)],
                         axis=0).reshape(BH, S)        # [BH, S] f32
    out = oT.astype(np.float32) / den[:, None, :]
    out = np.ascontiguousarray(out.transpose(0, 2, 1))  # [BH, S, D]
    out = out.reshape(B, H, S, D)
    if _trace:
        return out, res
    return out


# revision 45
# speedup vs baseline: 1.0471x; 1.0471x over previous
"""Causal scaled-dot-product attention for Trainium2 (Bass/Tile), 8-core SPMD.

Problem: B=2, H=16, S=2048, D=128 fp32, causal mask, softmax(QK^T/sqrt(D)) @ V.
Sharding: batch*heads (32) split across 8 cores, 4 heads per core; attention is
independent per (b,h): no communication.

Design (90.4us measured vs the 115.3us v1 baseline; rel err 9.8e-3):
  - All layout/dtype prep host-side. Q,K ship transposed [D,S] bf16 (fp8
    gives no PE speedup on TRN2 -- 1 col/cycle either way -- so bf16 keeps
    the accuracy for free); V ships fp8e4m3 partition-major plus a bf16 copy
    of its first 256 rows. Q/K DMAs are split into 512-column pieces so the
    first QK matmul can start as soon as ~256KB has landed.
  - exp is split across two engines so the Scalar engine never paces the
    pipeline:
      * most full (non-diagonal) pairs: ACT exp -> fp8e4m3 (exact path)
      * diagonal pairs + every 3rd full pair: DVE tensor_scalar computes
          y_int8 = round(psum * (4*log2e/T) + bias)
        and the int8 bytes ARE fp8e5m2 exp values (Schraudolph bit-trick,
        4 bytes/octave; the fp32->int8 convert rounds-to-nearest and
        saturates on HW). For diagonal pairs the bias comes from a constant
        maskbias tile: the exp bias on valid positions, -1000 on causally
        masked + stale positions, which saturates to int8 -128 = e5m2
        "-0.0" (harmless in the PV/den matmuls). One DVE op = exp + causal
        mask + stale kill. e5m2's byte window spans ~22 z-units: no wrap
        cliffs for any input.
  - PV per pair: ONE fp8 DoubleRow matmul (contraction 256), full width
    from the pair's first valid column -- masked/stale entries are -0.0 so
    no strip matmuls are needed. Mixed e4m3 weights x e5m2 moving verified
    on HW.
  - den matmuls for a whole chunk are emitted as one deferred batch (two
    pairs into the next chunk): consecutive DoubleRow matmuls sharing the
    constant ones weights stream at 1 col/cycle, where fresh-weight
    LDWEIGHTS cost ~190ns extra each (256-row DR weight loads do not
    double-buffer).
  - PSUM: ps_s [128,1024]x3 (6 banks) + ps_o [128,512] + ps_d [1,512].
    The 3-deep ps_s ring lets QK(g) proceed once exp(g-3) is done, which
    both deepens the HW pipeline and lets the Tile scheduler keep the PE
    stream dense.
  - No on-device softmax normalization: the kernel ships OUT^T (bf16,
    unnormalized) and den (f32); the host divides. This removes the fp32
    broadcast matmuls, the reciprocal chain, and the PE stalls at each
    chunk tail.

Numerics: softmax shift exp(z/T - 2) keeps the exact-path exp <= ~53 (no fp8
clipping); numerator and denominator consume the same quantized P~, so P
quantization largely cancels in the host-side normalization. First key-tile
pair of each head runs in bf16 (rows with <256 keys get no averaging of V's
fp8 quantization error). Measured worst rel err 9.8e-3 (tol 2e-2).
"""
import numpy as np

import concourse.bacc as bacc
import concourse.tile as tile
import concourse.mybir as mybir
from concourse.bass_utils import run_bass_kernel_spmd
from concourse.masks import make_upper_triangular

F32 = mybir.dt.float32
BF16 = mybir.dt.bfloat16
F8 = mybir.dt.float8e4
E5 = mybir.dt.float8e5
I8 = mybir.dt.int8
EXP = mybir.ActivationFunctionType.Exp
DR = mybir.MatmulPerfMode.DoubleRow

B, H, S, D = 2, 16, 2048, 128
TEMPERATURE = 11.313708498984761  # sqrt(128)
EXP_BIAS = -2.0          # exp(z/temp - 2): keeps exact-path exp <= ~53
A5 = 5.770780163555855   # 4*log2(e): e5m2 bytes per ln unit
C5 = 0.25                # Schraudolph round-to-nearest correction (tuned)
SCALE5 = A5 / TEMPERATURE
MB_VALID = 60.0 - C5 + A5 * EXP_BIAS   # fast-exp byte bias on valid entries
MB_MASK = -1000.0                      # masked -> int8 -128 -> e5m2 -0.0
N_CORES = 8
HEADS_PER_CORE = (B * H) // N_CORES  # 4
P = 128
CHUNK = 512
N_KT = S // P              # 16 key tiles per head
N_CH = S // CHUNK          # 4 query chunks per head


def build_attention_nc():
    nc = bacc.Bacc("TRN2", target_bir_lowering=False, debug=False,
                   num_devices=N_CORES)
    qT_d = nc.dram_tensor("qT", [HEADS_PER_CORE, D, S], BF16,
                          kind="ExternalInput").ap()
    kT_d = nc.dram_tensor("kT", [HEADS_PER_CORE, D, S], BF16,
                          kind="ExternalInput").ap()
    v8_d = nc.dram_tensor("v8", [HEADS_PER_CORE, P, N_KT, P], F8,
                          kind="ExternalInput").ap()
    vb_d = nc.dram_tensor("vb", [HEADS_PER_CORE, P, 2, P], BF16,
                          kind="ExternalInput").ap()
    mba_d = nc.dram_tensor("mba", [P, 2 * CHUNK], F32,
                           kind="ExternalInput").ap()
    mbb_d = nc.dram_tensor("mbb", [P, 2 * CHUNK], F32,
                           kind="ExternalInput").ap()
    o_d = nc.dram_tensor("oT", [HEADS_PER_CORE, D, S], BF16,
                         kind="ExternalOutput").ap()
    den_d = nc.dram_tensor("den", [HEADS_PER_CORE, N_CH, CHUNK], F32,
                           kind="ExternalOutput").ap()

    with tile.TileContext(nc) as tc:
        with tc.tile_pool(name="sb", bufs=1) as sb, \
             tc.tile_pool(name="ps_s", bufs=3, space="PSUM") as ps_s, \
             tc.tile_pool(name="ps_o", bufs=1, space="PSUM") as ps_o, \
             tc.tile_pool(name="ps_d", bufs=1, space="PSUM") as ps_d:
            consts = qkt = px = sm = sb

            # ---- constants ----
            utm = consts.tile([P, P], BF16)  # utm[k,q] = 1 iff q >= k
            make_upper_triangular(nc, utm, val=1.0, diag=True)
            ones_col = consts.tile([P, 1], BF16)
            nc.vector.memset(ones_col, 1.0)
            # fp8 ones pair for DoubleRow den matmuls ([128,2,1], 16B-aligned
            # pair stride per the DoubleRow weight AP requirement)
            ones8w = consts.tile([P, 2, 16], F8)
            nc.vector.memset(ones8w, 1.0)
            ones8 = ones8w[:, :, 0:1]
            wscr = consts.tile([P, CHUNK], BF16)
            nc.vector.memset(wscr, 1.0)
            bias_ap = consts.tile([P, 1], F32)
            nc.vector.memset(bias_ap, EXP_BIAS)
            mba = consts.tile([P, 2 * CHUNK], F32)
            mbb = consts.tile([P, 2 * CHUNK], F32)
            # preload the ACT exp table during the head-0 DMA (the implicit
            # ACT_TABLE_LOAD takes ~1.3us and would otherwise stall the
            # first real exp)
            actwarm = consts.tile([P, 1], F8)
            nc.scalar.activation(actwarm, bias_ap, EXP, bias=0.0, scale=1.0)

            head_state = {}

            def emit_load(hh, first_head=False):
                h = hh % HEADS_PER_CORE
                # split Q/K into 512-col pieces so chunk-0 work can start
                # before the whole head has landed
                kt = [qkt.tile([P, CHUNK], BF16, tag=f"kt{i}", name=f"kt{i}",
                               bufs=2) for i in range(4)]
                qc = [qkt.tile([P, CHUNK], BF16, tag=f"qc{i}", name=f"qc{i}",
                               bufs=2) for i in range(4)]
                v8 = qkt.tile([P, N_KT, P], F8, tag="v8", name="v8", bufs=2)
                vb = qkt.tile([P, 2, P], BF16, tag="vb", name="vb", bufs=2)
                def piece(t, i, dram):
                    nc.sync.dma_start(out=t[i],
                                      in_=dram[h, :, CHUNK * i:CHUNK * (i + 1)])
                if first_head:
                    # head 0 processes chunk 1 first (4 pairs of work vs
                    # chunk 0's 2, to prime the pipeline during the
                    # HAM-slow startup window): load its deps first
                    piece(kt, 0, kT_d)
                    piece(qc, 1, qT_d)
                    piece(kt, 1, kT_d)
                    nc.sync.dma_start(out=mba, in_=mba_d)
                    nc.sync.dma_start(out=mbb, in_=mbb_d)
                    piece(qc, 0, qT_d)
                    nc.sync.dma_start(out=vb, in_=vb_d[h])
                    nc.sync.dma_start(out=v8, in_=v8_d[h])
                    for i in (2, 3):
                        piece(qc, i, qT_d)
                        piece(kt, i, kT_d)
                else:
                    piece(kt, 0, kT_d)
                    piece(qc, 0, qT_d)
                    nc.sync.dma_start(out=vb, in_=vb_d[h])
                    nc.sync.dma_start(out=v8, in_=v8_d[h])
                    for i in range(1, 4):
                        piece(qc, i, qT_d)
                        piece(kt, i, kT_d)
                head_state[hh] = dict(kt=kt, qc=qc, v8=v8, vb=vb)

            emit_load(0, first_head=True)

            def emit_dummies(n):
                # real MAC activity to open the HAM clock gate / p-state
                # ramp. Covers BOTH ps_s ring slots over their full width so
                # every psum_s bit is initialized (bounded) before the
                # fast-exp path ever reads a stale region.
                for _ in range(n):
                    warm = ps_s.tile([P, 2 * CHUNK], F32, tag="psm",
                                     name="psm")
                    nc.tensor.matmul(warm[:, 0:CHUNK], wscr[:, 0:P], wscr,
                                     start=True, stop=True,
                                     skip_group_check=True)
                    nc.tensor.matmul(warm[:, CHUNK:2 * CHUNK], wscr[:, 0:P],
                                     wscr, start=True, stop=True,
                                     skip_group_check=True)

            def emit_pv_first(st, pexp, psum_o):
                # bf16 PV for the head's first pair (tiles 0,1): per-tile
                # matmuls with column offsets (skip the stale gap [512:640))
                for (j, oj) in ((0, 0), (1, P)):
                    base = j * CHUNK
                    nc.tensor.matmul(
                        psum_o[:, oj:CHUNK], st["vb"][:, j, :],
                        pexp[:, base + oj:base + CHUNK],
                        start=(j == 0), stop=False,
                        skip_group_check=True)

            def emit_pv(st, j0, oj0, pexp8, psum_o, start, stop):
                # one DoubleRow matmul pair over [oj0:CHUNK]; masked/stale
                # entries in pexp8 are (-)0.0 so the full width is safe
                p3 = pexp8.rearrange("p (a b) -> p a b", a=2)
                nc.tensor.matmul(
                    psum_o[:, oj0:CHUNK], st["v8"][:, j0:j0 + 2, :],
                    p3[:, :, oj0:CHUNK],
                    start=start, stop=stop,
                    perf_mode=DR, skip_group_check=True)

            def emit_chunk_pvs(st, chunk_pexps, psum_o):
                # all of a chunk's PV matmuls back-to-back: one PE
                # bf16<->fp8-DR mode transition per burst instead of two
                # per pair
                n = len(chunk_pexps)
                for i, (kind, pexp, oj0, j0) in enumerate(chunk_pexps):
                    if kind == "first":
                        emit_pv_first(st, pexp, psum_o)
                    else:
                        emit_pv(st, j0, oj0, pexp, psum_o,
                                start=(i == 0), stop=(i == n - 1))

            def emit_den_batch(chunk_pexps, psum_d, start=True, stop=True):
                # all of a chunk's den matmuls back-to-back: consecutive
                # DoubleRow matmuls sharing the constant ones weights stream
                # at 1 col/cycle (fresh-weight LDWEIGHTS would add ~190ns
                # per matmul otherwise)
                n = len(chunk_pexps)
                for i, (kind, pexp, oj0, j0) in enumerate(chunk_pexps):
                    if kind == "first":
                        for (j, oj) in ((0, 0), (1, P)):
                            base = j * CHUNK
                            nc.tensor.matmul(
                                psum_d[:, oj:CHUNK], ones_col,
                                pexp[:, base + oj:base + CHUNK],
                                start=(start and i == 0 and j == 0),
                                stop=False,
                                skip_group_check=True)
                    else:
                        p3 = pexp.rearrange("p (a b) -> p a b", a=2)
                        nc.tensor.matmul(
                            psum_d[:, oj0:CHUNK], ones8,
                            p3[:, :, oj0:CHUNK],
                            start=(start and i == 0),
                            stop=(stop and i == n - 1),
                            perf_mode=DR, skip_group_check=True)

            def make_tail(hh, c, psum_o, psum_d):
                def emit():
                    h = hh % HEADS_PER_CORE
                    outT = sm.tile([P, CHUNK], BF16, tag="outT", name="outT",
                                   bufs=3)
                    denb = sm.tile([1, CHUNK], F32, tag="denb", name="denb",
                                   bufs=3)
                    nc.scalar.copy(outT, psum_o)
                    nc.vector.tensor_copy(denb, psum_d)
                    nc.sync.dma_start(
                        out=o_d[h, :, CHUNK * c:CHUNK * (c + 1)], in_=outT)
                    nc.sync.dma_start(out=den_d[h, c:c + 1], in_=denb)
                return emit

            # ---- PE warm-up during the head-0 DMA ----
            # 3 iterations cover all 3 ps_s ring slots exactly
            emit_dummies(3)

            deferred = []           # FIFO of (due_group_idx, fn)
            group_idx = 0

            def pump(final=False):
                while deferred and (final or group_idx >= deferred[0][0]):
                    deferred.pop(0)[1]()

            def kw(st, j):
                # K^T weights for key tile j out of the split kT pieces
                return st["kt"][j // 4][:, (j % 4) * P:(j % 4 + 1) * P]

            for hh in range(HEADS_PER_CORE):
                st = head_state[hh]
                if hh + 1 < HEADS_PER_CORE:
                    emit_load(hh + 1)

                # head 0 runs chunk 1 before chunk 0: chunks are mutually
                # independent, and 4 pairs of up-front work keep the PE fed
                # while the pipeline (and HAM clock) warm up
                order = (1, 0, 2, 3) if hh == 0 else range(N_CH)
                for c in order:
                    last = (hh == HEADS_PER_CORE - 1 and c == N_CH - 1)
                    if last:
                        # clear the previous chunk's deferred den batch/tail
                        # before the eager last chunk touches psum_d/psum_o
                        pump(final=True)
                    jmax = 4 * c + 3
                    psum_o = ps_o.tile([P, CHUNK], F32, tag="po", name="po")
                    psum_d = ps_d.tile([1, CHUNK], F32, tag="pd", name="pd")
                    chunk_pexps = []

                    for jp in range(2 * c + 2):
                        j0 = 2 * jp
                        first = (c == 0 and jp == 0)
                        typeA = (j0 == 4 * c) and not first
                        typeB = (j0 == 4 * c + 2)
                        psum_s = ps_s.tile([P, 2 * CHUNK], F32, tag="psm",
                                           name="psm")

                        if first:
                            nc.tensor.matmul(
                                psum_s[:, 0:CHUNK], kw(st, 0), st["qc"][0],
                                start=True, stop=True)
                            nc.tensor.matmul(
                                psum_s[:, CHUNK + P:2 * CHUNK], kw(st, 1),
                                st["qc"][0][:, P:CHUNK],
                                start=True, stop=True)
                            pexp16 = px.tile([P, 2 * CHUNK], BF16,
                                             tag="pexp16", name="pexp16",
                                             bufs=2)
                            nc.scalar.activation(
                                pexp16, psum_s, EXP,
                                bias=bias_ap, scale=1.0 / TEMPERATURE)
                            # causal masks for the two diagonal blocks
                            nc.gpsimd.tensor_mul(
                                pexp16[:, 0:P], pexp16[:, 0:P], utm)
                            nc.gpsimd.tensor_mul(
                                pexp16[:, CHUNK + P:CHUNK + 2 * P],
                                pexp16[:, CHUNK + P:CHUNK + 2 * P], utm)
                            chunk_pexps.append(("first", pexp16, 0, 0))
                            if last:
                                emit_pv_first(st, pexp16, psum_o)
                        else:
                            oj0 = max(0, P * j0 - CHUNK * c)
                            oj1 = max(0, P * (j0 + 1) - CHUNK * c)
                            nc.tensor.matmul(
                                psum_s[:, oj0:CHUNK], kw(st, j0),
                                st["qc"][c][:, oj0:CHUNK],
                                start=True, stop=True)
                            nc.tensor.matmul(
                                psum_s[:, CHUNK + oj1:2 * CHUNK],
                                kw(st, j0 + 1),
                                st["qc"][c][:, oj1:CHUNK],
                                start=True, stop=True)
                            diag = typeA or typeB
                            if diag or jp % 3 == 2:
                                # DVE fast-exp -> e5m2 bytes; diagonal pairs
                                # add the fused causal mask via the maskbias
                                # tile, full pairs use an immediate bias
                                pexpd = px.tile([P, 2 * CHUNK], E5,
                                                tag="pexpd", name="pexpd",
                                                bufs=4)
                                pexp_i8 = pexpd.bitcast(I8)
                                if diag:
                                    mb = mba if typeA else mbb
                                    nc.vector.scalar_tensor_tensor(
                                        pexp_i8[:, oj0:2 * CHUNK],
                                        psum_s[:, oj0:2 * CHUNK], SCALE5,
                                        mb[:, oj0:2 * CHUNK],
                                        mybir.AluOpType.mult,
                                        mybir.AluOpType.add)
                                else:
                                    nc.vector.tensor_scalar(
                                        pexp_i8[:, 0:2 * CHUNK],
                                        psum_s[:, 0:2 * CHUNK], SCALE5,
                                        MB_VALID,
                                        mybir.AluOpType.mult,
                                        mybir.AluOpType.add)
                                pexp8 = pexpd
                            else:
                                # exact path: ACT exp -> fp8e4m3
                                pexp8 = px.tile([P, 2 * CHUNK], F8,
                                                tag="pexp8", name="pexp8",
                                                bufs=5)
                                nc.scalar.activation(
                                    pexp8, psum_s, EXP,
                                    bias=bias_ap, scale=1.0 / TEMPERATURE)
                            chunk_pexps.append(("pair", pexp8, oj0, j0))
                            if last:
                                emit_pv(st, j0, oj0, pexp8, psum_o,
                                        start=(j0 == 0),
                                        stop=(j0 + 1 == jmax))
                        group_idx += 1
                        pump()
                        if last:
                            # emit PV+den per pair eagerly on the very last
                            # chunk so the final flush isn't serialized
                            emit_den_batch(chunk_pexps[-1:], psum_d,
                                           start=(jp == 0),
                                           stop=(jp == 2 * c + 1))

                    # den batch + evac run two pairs into the NEXT chunk so
                    # the PE never waits on this chunk's last exp before
                    # starting the next chunk's QKs. With single-buffered
                    # ps_o/ps_d the evac MUST be emitted before the next
                    # chunk's first PV pops (at pair 3, lag 3) -- due+2 and
                    # FIFO order (batch, then tail) guarantee that.
                    pexps = list(chunk_pexps)
                    pd = psum_d
                    po = psum_o
                    if not last:
                        # (the last chunk emitted its PVs/dens per pair)
                        deferred.append((group_idx + 2,
                                         lambda ps=pexps, s=st, o=po, d=pd: (
                                             emit_chunk_pvs(s, ps, o),
                                             emit_den_batch(ps, d))))
                    deferred.append((group_idx + 3, make_tail(hh, c, psum_o,
                                                              psum_d)))

            pump(final=True)

    nc.compile()
    return nc


_NC_CACHE = None


def _get_nc():
    global _NC_CACHE
    if _NC_CACHE is None:
        _NC_CACHE = build_attention_nc()
    return _NC_CACHE


def _build_maskbias():
    """Constant [128,1024] f32 bias tiles for the two diagonal pair types.

    Pair layout: tile j0 at cols [0:512), tile j1 at cols [512:1024).
    Type A (oj0=0, oj1=128): masked at {col < p} in tile j0's diag block and
      cols [512, 640+p) (stale gap + tile j1 diag block).
    Type B (oj0=256, oj1=384): masked at cols [256, 256+p) and [512, 896+p).
    """
    pidx = np.arange(P)[:, None]
    cidx = np.arange(2 * CHUNK)[None, :]
    mba = np.where((cidx < pidx) | ((cidx >= 512) & (cidx < 640 + pidx)),
                   MB_MASK, MB_VALID).astype(np.float32)
    mbb = np.where((cidx < 256 + pidx) | ((cidx >= 512) & (cidx < 896 + pidx)),
                   MB_MASK, MB_VALID).astype(np.float32)
    return mba, mbb


def kernel(q, k, v, mask=None, _trace=False):
    """Full-input entry point: q,k,v [2,16,2048,128] f32, mask [2,1,2048,2048]
    int32 (causal; the kernel hardcodes causality and does not read it).
    Returns [2,16,2048,128] f32. Layout/dtype prep, the softmax
    normalization (out/den), and the inverse output transpose run host-side.
    """
    import ml_dtypes
    bf16 = ml_dtypes.bfloat16
    f8 = ml_dtypes.float8_e4m3fn

    nc = _get_nc()
    BH = B * H
    qf = np.asarray(q, dtype=np.float32).reshape(BH, S, D)
    kf = np.asarray(k, dtype=np.float32).reshape(BH, S, D)
    vf = np.asarray(v, dtype=np.float32).reshape(BH, S, D)
    qT = np.ascontiguousarray(qf.transpose(0, 2, 1)).astype(bf16)  # [BH,D,S]
    kT = np.ascontiguousarray(kf.transpose(0, 2, 1)).astype(bf16)
    # V partition-major: [BH, S, D] -> [BH, P, N_KT, D]
    v8 = np.ascontiguousarray(
        vf.reshape(BH, N_KT, P, D).transpose(0, 2, 1, 3)).astype(f8)
    vb = np.ascontiguousarray(
        vf[:, 0:2 * P].reshape(BH, 2, P, D).transpose(0, 2, 1, 3)).astype(bf16)
    mba, mbb = _build_maskbias()

    in_maps = []
    for i in range(N_CORES):
        sl = slice(i * HEADS_PER_CORE, (i + 1) * HEADS_PER_CORE)
        in_maps.append({"qT": qT[sl], "kT": kT[sl], "v8": v8[sl],
                        "vb": vb[sl], "mba": mba, "mbb": mbb})
    res = run_bass_kernel_spmd(nc, in_maps, list(range(N_CORES)), trace=_trace)
    oT = np.concatenate([res.results[i]["oT"] for i in range(N_CORES)],
                        axis=0)                        # [BH, D, S] bf16
    den = np.concatenate([res.results[i]["den"] for i in range(N_CORES)],
                         axis=0).reshape(BH, S)        # [BH, S] f32
    out = oT.astype(np.float32) / den[:, None, :]
    out = np.ascontiguousarray(out.transpose(0, 2, 1))  # [BH, S, D]
    out = out.reshape(B, H, S, D)
    if _trace:
        return out, res
    return out
